# revision 1
# baseline (speedup 1.0000x reference)
"""Trainium2 Bass kernel for nn_Block_with_lora (dense transformer block).

Sharding: 8 cores = 4 batches x 2 token-parity shards (stride-2 over T).
Each core computes its 512 query tokens end-to-end (no collectives);
K/V projections over all 1024 tokens are computed per-core (uniform SPMD
program; all batch/parity dependence lives in the per-core input data).

Layout: all activations transposed [C, T] (host transposes I/O), so every
projection is a natural PE matmul. Attention uses S^T = K^T.T @ Q^T tiles
[tk, tq]; softmax denominator rides the AV matmul as an extra ones-column
of V; masking = additive diagonal band (DVE) + rectangle memsets (GPSIMD).
"""

import sys

sys.path.insert(0, "/opt/trn_rl_repo")

import numpy as np
import ml_dtypes
from contextlib import ExitStack

BF = ml_dtypes.bfloat16

C = 1024
H = 16
DH = 64
R = 16
SCALE = 1.0 / R
T = 1024
TQ = 512
NT = 8  # C / 128
EPS = 1e-5
NCORES = 8

_PROG = None


def _build_program():
    import concourse.bass as bass
    import concourse.tile as tile
    from concourse import mybir, bacc

    f32 = mybir.dt.float32
    bf16 = mybir.dt.bfloat16
    AF = mybir.ActivationFunctionType
    AL = mybir.AluOpType

    nc = bacc.Bacc("TRN2", target_bir_lowering=False, debug=False)

    def din(name, shape, dt=f32):
        return nc.dram_tensor(name, shape, dt, kind="ExternalInput").ap()

    xT_d = din("xT", [C, T])
    xqT_d = din("xqT", [C, TQ])
    fT_d = din("fT", [C, T])
    band_d = din("band", [128, 64])

    w_d = {}
    for n in ["wq", "wk", "wv", "wsp", "wcq", "wck", "wcv", "wcp"]:
        w_d[n] = din(n, [C, C], bf16)
    w_d["wfc"] = din("wfc", [C, 4 * C], bf16)
    w_d["wpr"] = din("wpr", [4 * C, C], bf16)
    a_d = {n: din(n, [C, R], bf16) for n in ["a_sa", "a_sp", "a_cq", "a_ck", "a_cp"]}
    b_d = {
        n: din(n, [R, C], bf16)
        for n in ["b_saq", "b_sak", "b_sav", "b_sp", "b_cq", "b_ckk", "b_ckv", "b_cp"]
    }
    bias_d = {
        n: din(n, [C], f32)
        for n in ["bq", "bk", "bsp", "bcq", "bck", "bcp", "bpr", "g1", "b1", "g2", "b2"]
    }
    bias_d["bfc"] = din("bfc", [4 * C], f32)
    bvrow_d = din("bv_row", [1, C], bf16)
    bckrow_d = din("bck_row", [1, C], bf16)
    sel_d = din("sel", [NT, R, 128], f32)
    bcvrow_d = din("bcv_row", [1, C], bf16)

    outT_d = nc.dram_tensor("outT", [C, TQ], f32, kind="ExternalOutput").ap()

    with tile.TileContext(nc) as tc, ExitStack() as ctx:

        def pool(name, bufs, space=None):
            kw = dict(name=name, bufs=bufs)
            if space:
                kw["space"] = space
            return ctx.enter_context(tc.tile_pool(**kw))

        # SBUF pools (budget ~181KB/partition of 192)
        big32 = pool("big32", 2)        # [128,1024] f32: x/f stream + LN temps
        acts = pool("acts", 8)          # [128,1024] bf16: lnb then fb
        lnsm = pool("lnsm", 8)          # [128,512] bf16: lnown -> ln1b -> ln2
        qpool = pool("qpool", 8)        # [128,512] bf16: qT -> q2T
        kpool = pool("kpool", 8)        # [128,1024] bf16: kT
        k2pool = pool("k2pool", 8)      # [128,1024] bf16: k2T (separate: overlaps attn)
        vpool = pool("vpool", 8)        # [128,1040] bf16: V -> V2
        opool = pool("opool", 8)        # [128,512] bf16: oT -> o2T
        rpool = pool("rpool", 8)        # [128,512] f32: residual (persist)
        mpool = pool("mpool", 32)       # [128,256] bf16: MLP hidden (per t-half)
        wpool = pool("wpool", 10)        # [128,512] bf16: weight chunks
        epool = pool("epool", 3)        # [128,1024] bf16: exp(S)
        sqpool = pool("sqpool", 3)      # squares for LN var
        sbig = pool("sbig", 2)          # [128,1024] f32: LN mean/rstd bcast
        rows = pool("rows", 2)          # [1,1024] f32: LN stat rows
        rrows = pool("rrows", 2)        # [1,512] f32: softmax recip rows
        recb = pool("recb", 2)          # [64,512] f32: recip bcast
        dallp = pool("dallp", 2)        # [16,512] f32: batched softmax denoms
        outfp = pool("outfp", 2)        # [128,256] f32: final out staging
        zpool = pool("zpool", 1)        # [16,*] bf16: lora z (1 slot per tag)
        lorab = pool("lorab", 1)        # [16,1024] bf16: lora B rows
        loraa = pool("loraa", 10)       # [128,16] bf16: lora A chunks
        smalls = pool("smalls", 1)      # [128,<=32] bias/g/b columns (per tag)
        onesp = pool("onesp", 1)
        bandp = pool("bandp", 1)
        bvp = pool("bvp", 1)            # [1,1024] bf16 v-bias rows

        # PSUM pools: 4 + 2 + 2 = 8 banks
        ps = pool("ps", 2, space="PSUM")   # [128,1024] f32: S tiles, LN stats, pr acc
        po = pool("po", 2, space="PSUM")   # [65..128,512] f32: attn out acc, pr acc
        pp = pool("pp", 2, space="PSUM")   # [128,512] f32: projections, z

        # ---- constants ----
        ones_c32 = onesp.tile([128, 1], f32, tag="oc32")
        nc.gpsimd.memset(ones_c32[:], 1.0)
        ones_c16 = onesp.tile([128, 1], bf16, tag="oc16")
        nc.gpsimd.memset(ones_c16[:], 1.0)
        ones_r16 = onesp.tile([1, 128], bf16, tag="or16")
        nc.gpsimd.memset(ones_r16[:], 1.0)
        ones_r32 = onesp.tile([1, 128], f32, tag="or32")
        nc.gpsimd.memset(ones_r32[:], 1.0)
        ones_row512 = onesp.tile([1, 512], bf16, tag="or512")
        nc.gpsimd.memset(ones_row512[:], 1.0)

        band_t = bandp.tile([128, 64], f32, tag="band")
        nc.sync.dma_start(band_t[:], band_d[:, :])
        # selector matrices: sel[mi] @ dall broadcasts head 2mi to rows 0:64
        # and head 2mi+1 to rows 64:128 (softmax denominator rescale)
        sel_t = []
        for mi in range(NT):
            st_ = smalls.tile([R, 128], f32, tag=f"sel{mi}", name=f"sel{mi}")
            nc.sync.dma_start(st_[:], sel_d[mi])
            sel_t.append(st_)
        eps_t = onesp.tile([1, 1], f32, tag="eps")
        nc.gpsimd.memset(eps_t[:], EPS)

        dma_rr = [0]
        def wdma(dst, src):
            # spread weight streaming across two DMA queues
            eng = (nc.sync, nc.gpsimd)[dma_rr[0] % 2]
            dma_rr[0] += 1
            eng.dma_start(dst, src)

        def load_percol(name, n=NT):
            t = smalls.tile([128, n], f32, tag=name)
            nc.sync.dma_start(t[:], bias_d[name].rearrange("(m p) -> p m", p=128))
            return t

        bias_t = {
            n: load_percol(n)
            for n in ["bq", "bk", "bsp", "bcq", "bcp", "bpr", "g1", "b1", "g2", "b2", "bck"]
        }
        bias_t["bfc"] = load_percol("bfc", 32)
        bv_t = bvp.tile([1, C], bf16, tag="bv")
        nc.sync.dma_start(bv_t[:], bvrow_d[:, :])
        bcv_t = bvp.tile([1, C], bf16, tag="bcv")
        nc.sync.dma_start(bcv_t[:], bcvrow_d[:, :])
        bck_row_t = bvp.tile([1, C], bf16, tag="bckr")
        nc.sync.dma_start(bck_row_t[:], bckrow_d[:, :])

        def load_lora_a(name):
            ts = []
            for k in range(NT):
                t = loraa.tile([128, R], bf16, tag="loraa")
                nc.sync.dma_start(t[:], a_d[name][k * 128:(k + 1) * 128, :])
                ts.append(t)
            return ts

        def load_lora_b(name):
            t = lorab.tile([R, C], bf16, tag="lorab")
            nc.sync.dma_start(t[:], b_d[name][:, :])
            return t

        # =============== helpers ===============
        def bcast_row(row, out_sb, Tn):
            # broadcast [1, Tn] f32 row to [128, Tn] SBUF via K=1 PE matmul
            for h in range(Tn // 512):
                sl = slice(h * 512, (h + 1) * 512)
                bp = pp.tile([128, 512], f32, tag="pp")
                nc.tensor.matmul(bp[:], ones_r32[:], row[0:1, sl], start=True, stop=True)
                nc.vector.tensor_copy(out_sb[:, sl], bp[:])

        def ln_stats_and_norm(src_tiles, g_col, b_col, out_tiles):
            """LayerNorm over channel (partition) dim; src 8x[128,512] f32 persistent."""
            mean_ps = ps.tile([1, TQ], f32, tag="ps")
            sq_ps = ps.tile([1, TQ], f32, tag="ps")
            for k in range(NT):
                xb = sqpool.tile([128, TQ], bf16, tag="sqo")
                nc.vector.tensor_copy(xb[:], src_tiles[k][:])
                sq = sqpool.tile([128, TQ], bf16, tag="sqo")
                nc.vector.tensor_mul(sq[:], xb[:], xb[:])
                nc.tensor.matmul(mean_ps[:], ones_c16[:], xb[:],
                                 start=(k == 0), stop=(k == NT - 1))
                nc.tensor.matmul(sq_ps[:], ones_c16[:], sq[:],
                                 start=(k == 0), stop=(k == NT - 1))
            mean_row = rows.tile([1, TQ], f32, tag="rows")
            rstd_row = rows.tile([1, TQ], f32, tag="rows")
            nc.vector.tensor_scalar_mul(mean_row[:], mean_ps[:], 1.0 / C)
            nc.vector.tensor_mul(rstd_row[:], mean_row[:], mean_row[:])
            nc.vector.scalar_tensor_tensor(rstd_row[:], sq_ps[:], 1.0 / C, rstd_row[:],
                                           op0=AL.mult, op1=AL.subtract)
            nc.scalar.activation(rstd_row[:], rstd_row[:], AF.Sqrt, bias=eps_t[:])
            nc.vector.reciprocal(rstd_row[:], rstd_row[:])
            mb = sbig.tile([128, TQ], f32, tag="sbig")
            rb = sbig.tile([128, TQ], f32, tag="sbig")
            bcast_row(mean_row, mb, TQ)
            bcast_row(rstd_row, rb, TQ)
            for k in range(NT):
                t1 = big32.tile([128, TQ], f32, tag="big32")
                nc.vector.tensor_sub(t1[:], src_tiles[k][:], mb[:])
                nc.vector.tensor_mul(t1[:], t1[:], rb[:])
                nc.scalar.activation(out_tiles[k][:], t1[:], AF.Identity,
                                     bias=b_col[:, k:k + 1], scale=g_col[:, k:k + 1])

        def compute_z(a_tiles, rhs_tiles, Tn, tag):
            """z^T = A-proj of activations: [16, Tn] bf16."""
            z_sb = zpool.tile([R, Tn], bf16, tag=tag)
            for h in range(Tn // 512):
                sl = slice(h * 512, (h + 1) * 512)
                zp = pp.tile([R, 512], f32, tag="pp")
                for k in range(NT):
                    nc.tensor.matmul(zp[:], a_tiles[k][:], rhs_tiles[k][:, sl],
                                     start=(k == 0), stop=(k == NT - 1))
                nc.vector.tensor_copy(z_sb[:, sl], zp[:])
            return z_sb

        def projT(wname, rhs_tiles, Tn, z_sb, bname, out_cb, pools=None,
                  bias_row_t=None):
            """out^T tiles via PE; lora + callback per (M-tile, t-half) psum."""
            if pools is None:
                pools = ((pp, "pp"),)
            b_t = load_lora_b(bname)
            pcnt = 0
            for mh in range(2):  # c_out halves of 512
                wts = []
                for k in range(NT):
                    wt = wpool.tile([128, 512], bf16, tag="wpool")
                    wdma(wt[:], w_d[wname][k * 128:(k + 1) * 128,
                                           mh * 512:(mh + 1) * 512])
                    wts.append(wt)
                for ml in range(4):
                    mi = mh * 4 + ml
                    for h in range(Tn // 512):
                        sl = slice(h * 512, (h + 1) * 512)
                        pl, ptag = pools[pcnt % len(pools)]
                        pcnt += 1
                        pt = pl.tile([128, 512], f32, tag=ptag)
                        for k in range(NT):
                            nc.tensor.matmul(pt[:], wts[k][:, ml * 128:(ml + 1) * 128],
                                             rhs_tiles[k][:, sl], start=(k == 0), stop=False)
                        if bias_row_t is not None:
                            nc.tensor.matmul(pt[:], bias_row_t[0:1, mi * 128:(mi + 1) * 128],
                                             ones_row512[:], start=False, stop=False)
                        nc.tensor.matmul(pt[:], b_t[:, mi * 128:(mi + 1) * 128],
                                         z_sb[:, sl], start=False, stop=True)
                        out_cb(mi, pt, h)

        def proj_V(wname, lhs_tiles, z_sb, bv_row_t, bname, v_tiles, pools=None):
            """V natural [t, d] with activations stationary; +lora +bias(ones-MM)."""
            if pools is None:
                pools = ((pp, "pp"),)
            b_t = load_lora_b(bname)
            pcnt = 0
            for dh in range(2):
                sl = slice(dh * 512, (dh + 1) * 512)
                wts = []
                for k in range(NT):
                    wt = wpool.tile([128, 512], bf16, tag="wpool")
                    wdma(wt[:], w_d[wname][k * 128:(k + 1) * 128, sl])
                    wts.append(wt)
                for tt in range(NT):
                    pl, ptag = pools[pcnt % len(pools)]
                    pcnt += 1
                    pt = pl.tile([128, 512], f32, tag=ptag)
                    for k in range(NT):
                        nc.tensor.matmul(pt[:], lhs_tiles[k][:, tt * 128:(tt + 1) * 128],
                                         wts[k][:], start=(k == 0), stop=False)
                    nc.tensor.matmul(pt[:], z_sb[:, tt * 128:(tt + 1) * 128],
                                     b_t[:, sl], start=False, stop=False)
                    nc.tensor.matmul(pt[:], ones_r16[:], bv_row_t[:, sl],
                                     start=False, stop=True)
                    dest = v_tiles[tt][:, dh * 520:(dh + 1) * 520]
                    dest = dest.rearrange("p (h d) -> p h d", d=65)[:, :, 0:64]
                    nc.vector.tensor_copy(dest, pt[:])

        def attention(q_tiles, k_tiles, v_tiles, o_tiles):
            # Block kj only matters for queries i >= 64*kj (strided parity
            # layout), so every matmul/exp runs on the live tq-subrange.
            dall = dallp.tile([R, 512], f32, tag="dallp")
            for h in range(H):
                mi, off = h // 2, 64 * (h % 2)
                op = po.tile([65, 512], f32, tag="po")
                for jp in range(4):
                    # two key blocks per psum pair tile; live score regions
                    # packed contiguously so one Exp covers both
                    kj0, kj1 = 2 * jp, 2 * jp + 1
                    q0, q1 = 64 * kj0, 64 * kj1
                    e1 = 512 + (512 - q1)
                    st = ps.tile([128, 1024], f32, tag="ps")
                    nc.tensor.matmul(
                        st[:, q0:512],
                        k_tiles[mi][off:off + 64, kj0 * 128:(kj0 + 1) * 128],
                        q_tiles[mi][off:off + 64, q0:512],
                        start=True, stop=True)
                    nc.tensor.matmul(
                        st[:, 512:e1],
                        k_tiles[mi][off:off + 64, kj1 * 128:(kj1 + 1) * 128],
                        q_tiles[mi][off:off + 64, q1:512],
                        start=True, stop=True)
                    # additive causal band on the diagonal-straddling queries
                    nc.vector.tensor_add(st[:, q0:q0 + 64], st[:, q0:q0 + 64], band_t[:])
                    nc.vector.tensor_add(st[:, 512:576], st[:, 512:576], band_t[:])
                    et = epool.tile([128, 1024], bf16, tag="epool")
                    nc.scalar.activation(et[:, q0:e1], st[:, q0:e1], AF.Exp)
                    nc.tensor.matmul(
                        op[:] if kj0 == 0 else op[:, q0:512],
                        v_tiles[kj0][:, 65 * h:65 * h + 65],
                        et[:, q0:512], start=(kj0 == 0), stop=False)
                    nc.tensor.matmul(
                        op[:, q1:512],
                        v_tiles[kj1][:, 65 * h:65 * h + 65],
                        et[:, 512:e1], start=False, stop=(kj1 == 7))
                # stash raw (unnormalized) head output + denominator row
                nc.vector.tensor_copy(o_tiles[mi][off:off + 64, :], op[0:64, :])
                rr = rrows.tile([1, 512], f32, tag="rrows")
                nc.vector.tensor_copy(rr[:], op[64:65, :])
                nc.sync.dma_start(dall[h:h + 1, :], rr[:])
            # one batched reciprocal for all 16 heads, then per-tile rescale
            nc.vector.reciprocal(dall[:], dall[:])
            for mi2 in range(NT):
                bp = pp.tile([128, 512], f32, tag="pp")
                nc.tensor.matmul(bp[:], sel_t[mi2][:], dall[:], start=True, stop=True)
                rbc = recb.tile([128, 512], f32, tag="recb")
                nc.vector.tensor_copy(rbc[:], bp[:])
                nc.vector.tensor_mul(o_tiles[mi2][:], o_tiles[mi2][:], rbc[:])

        # =============== phase 1: LN1 over full x (2-pass) + own x ===============
        mean_ps = ps.tile([1, T], f32, tag="ps")
        sq_ps = ps.tile([1, T], f32, tag="ps")
        for k in range(NT):
            xt = big32.tile([128, T], f32, tag="big32")
            nc.sync.dma_start(xt[:], xT_d[k * 128:(k + 1) * 128, :])
            xb = sqpool.tile([128, T], bf16, tag="sqf")
            nc.vector.tensor_copy(xb[:], xt[:])
            sq = sqpool.tile([128, T], bf16, tag="sqf")
            nc.vector.tensor_mul(sq[:], xb[:], xb[:])
            for hh in range(2):
                sl = slice(hh * 512, (hh + 1) * 512)
                nc.tensor.matmul(mean_ps[0:1, sl], ones_c16[:], xb[:, sl],
                                 start=(k == 0), stop=(k == NT - 1))
                nc.tensor.matmul(sq_ps[0:1, sl], ones_c16[:], sq[:, sl],
                                 start=(k == 0), stop=(k == NT - 1))
        mean_row = rows.tile([1, T], f32, tag="rows")
        rstd_row = rows.tile([1, T], f32, tag="rows")
        nc.vector.tensor_scalar_mul(mean_row[:], mean_ps[:], 1.0 / C)
        nc.vector.tensor_mul(rstd_row[:], mean_row[:], mean_row[:])
        nc.vector.scalar_tensor_tensor(rstd_row[:], sq_ps[:], 1.0 / C, rstd_row[:],
                                       op0=AL.mult, op1=AL.subtract)
        nc.scalar.activation(rstd_row[:], rstd_row[:], AF.Sqrt, bias=eps_t[:])
        nc.vector.reciprocal(rstd_row[:], rstd_row[:])
        mb_f = sbig.tile([128, T], f32, tag="sbig")
        rb_f = sbig.tile([128, T], f32, tag="sbig")
        bcast_row(mean_row, mb_f, T)
        bcast_row(rstd_row, rb_f, T)
        lnb = [acts.tile([128, T], bf16, tag="acts", name=f"lnb{i}") for i in range(NT)]
        for k in range(NT):
            xt = big32.tile([128, T], f32, tag="big32")
            nc.sync.dma_start(xt[:], xT_d[k * 128:(k + 1) * 128, :])
            nc.vector.tensor_sub(xt[:], xt[:], mb_f[:])
            nc.vector.tensor_mul(xt[:], xt[:], rb_f[:])
            nc.scalar.activation(lnb[k][:], xt[:], AF.Identity,
                                 bias=bias_t["b1"][:, k:k + 1], scale=bias_t["g1"][:, k:k + 1])

        # own-token x -> residual tiles + LN(own)
        resid = []
        for k in range(NT):
            rt = rpool.tile([128, TQ], f32, tag="rpool")
            nc.sync.dma_start(rt[:], xqT_d[k * 128:(k + 1) * 128, :])
            resid.append(rt)
        lnown = [lnsm.tile([128, TQ], bf16, tag="lnsm", name=f"lnown{i}") for i in range(NT)]
        ln_stats_and_norm(resid, bias_t["g1"], bias_t["b1"], lnown)

        # =============== phase 2: self qkv ===============
        a_sa_t = load_lora_a("a_sa")
        z_sa = compute_z(a_sa_t, lnb, T, "zbig")
        z_own = compute_z(a_sa_t, lnown, TQ, "zsm")

        qT = [qpool.tile([128, TQ], bf16, tag="qpool", name=f"qT{i}") for i in range(NT)]

        def q_cb(mi, pt, h):
            nc.scalar.activation(qT[mi][:], pt[:], AF.Identity,
                                 bias=bias_t["bq"][:, mi:mi + 1])

        projT("wq", lnown, TQ, z_own, "b_saq", q_cb, pools=((pp, "pp"), (po, "po"), (ps, "ps")))

        kT = [kpool.tile([128, T], bf16, tag="kpool", name=f"kT{i}") for i in range(NT)]

        def k_cb(mi, pt, h):
            nc.scalar.activation(kT[mi][:, h * 512:(h + 1) * 512], pt[:], AF.Identity,
                                 bias=bias_t["bk"][:, mi:mi + 1])

        projT("wk", lnb, T, z_sa, "b_sak", k_cb, pools=((pp, "pp"), (po, "po"), (ps, "ps")))

        vt = [vpool.tile([128, 1040], bf16, tag="vpool", name=f"vt{i}") for i in range(NT)]
        for tt in range(NT):
            nc.gpsimd.memset(vt[tt][:, 64:1040:65], 1.0)
        proj_V("wv", lnb, z_sa, bv_t, "b_sav", vt, pools=((pp, "pp"), (po, "po"), (ps, "ps")))

        # =============== phase 3: cross K (PE filler during self-attn) ===============
        fb = [acts.tile([128, T], bf16, tag="acts", name=f"fb{i}") for i in range(NT)]
        for k in range(NT):
            ft = big32.tile([128, T], f32, tag="big32")
            nc.gpsimd.dma_start(ft[:], fT_d[k * 128:(k + 1) * 128, :])
            nc.vector.tensor_copy(fb[k][:], ft[:])
        a_ck_t = load_lora_a("a_ck")
        z_ck = compute_z(a_ck_t, fb, T, "zbig2")
        k2T = [k2pool.tile([128, T], bf16, tag="k2pool", name=f"k2T{i}") for i in range(NT)]

        def k2_cb(mi, pt, h):
            # DVE drain (bias already folded in via ones-matmul) keeps the
            # Scalar engine free for self-attention Exp
            nc.vector.tensor_copy(k2T[mi][:, h * 512:(h + 1) * 512], pt[:])

        projT("wck", fb, T, z_ck, "b_ckk", k2_cb, bias_row_t=bck_row_t)

        # =============== phase 4: self attention ===============
        oT = [opool.tile([128, TQ], bf16, tag="opool", name=f"oT{i}") for i in range(NT)]
        attention(qT, kT, vt, oT)

        # =============== phase 5: cross V (reuses V slots) ===============
        v2t = [vpool.tile([128, 1040], bf16, tag="vpool", name=f"v2t{i}") for i in range(NT)]
        for tt in range(NT):
            nc.gpsimd.memset(v2t[tt][:, 64:1040:65], 1.0)
        proj_V("wcv", fb, z_ck, bcv_t, "b_ckv", v2t, pools=((pp, "pp"), (po, "po")))

        # =============== phase 6: self proj + residual ===============
        a_sp_t = load_lora_a("a_sp")
        z_sp = compute_z(a_sp_t, oT, TQ, "zsm")

        def sp_cb(mi, pt, h):
            nc.vector.scalar_tensor_tensor(resid[mi][:], pt[:], bias_t["bsp"][:, mi:mi + 1],
                                           resid[mi][:], op0=AL.add, op1=AL.add)

        projT("wsp", oT, TQ, z_sp, "b_sp", sp_cb, pools=((pp, "pp"), (po, "po"), (ps, "ps")))

        # =============== phase 7: LN1 on updated own tokens ===============
        ln1b = [lnsm.tile([128, TQ], bf16, tag="lnsm", name=f"ln1b{i}") for i in range(NT)]
        ln_stats_and_norm(resid, bias_t["g1"], bias_t["b1"], ln1b)

        # =============== phase 8: cross q ===============
        a_cq_t = load_lora_a("a_cq")
        z_cq = compute_z(a_cq_t, ln1b, TQ, "zsm")
        q2T = [qpool.tile([128, TQ], bf16, tag="qpool", name=f"q2T{i}") for i in range(NT)]

        def q2_cb(mi, pt, h):
            nc.scalar.activation(q2T[mi][:], pt[:], AF.Identity,
                                 bias=bias_t["bcq"][:, mi:mi + 1])

        projT("wcq", ln1b, TQ, z_cq, "b_cq", q2_cb, pools=((pp, "pp"), (po, "po"), (ps, "ps")))

        # =============== phase 9: cross attention ===============
        o2T = [opool.tile([128, TQ], bf16, tag="opool", name=f"o2T{i}") for i in range(NT)]
        attention(q2T, k2T, v2t, o2T)

        # =============== phase 10: cross proj + residual ===============
        a_cp_t = load_lora_a("a_cp")
        z_cp = compute_z(a_cp_t, o2T, TQ, "zsm")

        def cp_cb(mi, pt, h):
            nc.vector.scalar_tensor_tensor(resid[mi][:], pt[:], bias_t["bcp"][:, mi:mi + 1],
                                           resid[mi][:], op0=AL.add, op1=AL.add)

        projT("wcp", o2T, TQ, z_cp, "b_cp", cp_cb, pools=((pp, "pp"), (po, "po"), (ps, "ps")))

        # =============== phase 11: LN2 + MLP (per token-half) ===============
        ln2 = [lnsm.tile([128, TQ], bf16, tag="lnsm", name=f"ln2_{i}") for i in range(NT)]
        ln_stats_and_norm(resid, bias_t["g2"], bias_t["b2"], ln2)

        for th in range(2):
            tsl = slice(th * 256, (th + 1) * 256)
            m_sb = [None] * 32
            for grp in range(8):
                wts = []
                for k in range(NT):
                    wt = wpool.tile([128, 512], bf16, tag="wpool")
                    wdma(wt[:], w_d["wfc"][k * 128:(k + 1) * 128,
                                           grp * 512:(grp + 1) * 512])
                    wts.append(wt)
                for ml in range(4):
                    mi = grp * 4 + ml
                    pl, ptag = ((pp, "pp"), (ps, "ps"))[ml % 2]
                    pt = pl.tile([128, 256], f32, tag=ptag)
                    for k in range(NT):
                        nc.tensor.matmul(pt[:], wts[k][:, ml * 128:(ml + 1) * 128],
                                         ln2[k][:, tsl], start=(k == 0), stop=(k == NT - 1))
                    mt = mpool.tile([128, 256], bf16, tag="mpool")
                    nc.scalar.activation(mt[:], pt[:], AF.Gelu_apprx_tanh,
                                         bias=bias_t["bfc"][:, mi:mi + 1])
                    m_sb[mi] = mt

            for quad in range(2):
                qts = []
                for j in range(4):
                    p_ = ps if j < 2 else po
                    qts.append(p_.tile([128, 256], f32, tag="ps" if j < 2 else "po", name=f"prq{th}_{quad}_{j}"))
                for k in range(32):
                    wt = wpool.tile([128, 512], bf16, tag="wpool")
                    wdma(wt[:], w_d["wpr"][k * 128:(k + 1) * 128,
                                           quad * 512:(quad + 1) * 512])
                    for j in range(4):
                        nc.tensor.matmul(qts[j][:], wt[:, j * 128:(j + 1) * 128],
                                         m_sb[k][:], start=(k == 0), stop=(k == 31))
                for j in range(4):
                    mi = quad * 4 + j
                    of = outfp.tile([128, 256], f32, tag="outfp")
                    nc.vector.scalar_tensor_tensor(of[:], qts[j][:],
                                                   bias_t["bpr"][:, mi:mi + 1],
                                                   resid[mi][:, tsl],
                                                   op0=AL.add, op1=AL.add)
                    nc.sync.dma_start(outT_d[mi * 128:(mi + 1) * 128, tsl], of[:])

    nc.compile()
    return nc


def _get_program():
    global _PROG
    if _PROG is None:
        _PROG = _build_program()
    return _PROG


def _prep_shared(inputs):
    g = {}

    def bf(a):
        return np.ascontiguousarray(np.asarray(a, dtype=np.float32)).astype(BF)

    def f(a):
        return np.ascontiguousarray(np.asarray(a, dtype=np.float32))

    qw, kw, vw = (inputs["sa_qkv_w"][i * C:(i + 1) * C] for i in range(3))
    qb, kb, vb = (inputs["sa_qkv_b"][i * C:(i + 1) * C] for i in range(3))
    qlb, klb, vlb = (inputs["sa_qkv_lb"][i * C:(i + 1) * C] for i in range(3))
    inv = 1.0 / np.sqrt(DH)
    g["wq"] = bf(np.asarray(qw).T * inv)
    g["wk"] = bf(np.asarray(kw).T)
    g["wv"] = bf(np.asarray(vw).T)
    g["bq"] = f(np.asarray(qb) * inv)
    g["bk"] = f(kb)
    g["bv_row"] = bf(np.asarray(vb).reshape(1, C))
    g["a_sa"] = bf(np.asarray(inputs["sa_qkv_a"]).T)
    g["b_saq"] = bf(np.asarray(qlb).T * (SCALE * inv))
    g["b_sak"] = bf(np.asarray(klb).T * SCALE)
    g["b_sav"] = bf(np.asarray(vlb).T * SCALE)

    g["wsp"] = bf(np.asarray(inputs["sa_proj_w"]).T)
    g["bsp"] = f(inputs["sa_proj_b"])
    g["a_sp"] = bf(np.asarray(inputs["sa_proj_a"]).T)
    g["b_sp"] = bf(np.asarray(inputs["sa_proj_lb"]).T * SCALE)

    g["wcq"] = bf(np.asarray(inputs["ca_q_w"]).T * inv)
    g["bcq"] = f(np.asarray(inputs["ca_q_b"]) * inv)
    g["a_cq"] = bf(np.asarray(inputs["ca_q_a"]).T)
    g["b_cq"] = bf(np.asarray(inputs["ca_q_lb"]).T * (SCALE * inv))

    ckw, cvw = inputs["ca_kv_w"][0:C], inputs["ca_kv_w"][C:2 * C]
    ckb, cvb = inputs["ca_kv_b"][0:C], inputs["ca_kv_b"][C:2 * C]
    cklb, cvlb = inputs["ca_kv_lb"][0:C], inputs["ca_kv_lb"][C:2 * C]
    g["wck"] = bf(np.asarray(ckw).T)
    g["wcv"] = bf(np.asarray(cvw).T)
    g["bck"] = f(ckb)
    g["bck_row"] = bf(np.asarray(ckb).reshape(1, C))
    g["bcv_row"] = bf(np.asarray(cvb).reshape(1, C))
    g["a_ck"] = bf(np.asarray(inputs["ca_kv_a"]).T)
    g["b_ckk"] = bf(np.asarray(cklb).T * SCALE)
    g["b_ckv"] = bf(np.asarray(cvlb).T * SCALE)

    g["wcp"] = bf(np.asarray(inputs["ca_proj_w"]).T)
    g["bcp"] = f(inputs["ca_proj_b"])
    g["a_cp"] = bf(np.asarray(inputs["ca_proj_a"]).T)
    g["b_cp"] = bf(np.asarray(inputs["ca_proj_lb"]).T * SCALE)

    g["wfc"] = bf(np.asarray(inputs["fc_w"]).T)
    g["bfc"] = f(inputs["fc_b"])
    g["wpr"] = bf(np.asarray(inputs["pr_w"]).T)
    g["bpr"] = f(inputs["pr_b"])
    g["g1"] = f(inputs["ln1_g"])
    g["b1"] = f(inputs["ln1_b"])
    g["g2"] = f(inputs["ln2_g"])
    g["b2"] = f(inputs["ln2_b"])
    return g


def _make_in_maps(inputs):
    inputs = {k: np.asarray(v) for k, v in inputs.items()}
    x, feat = inputs["x"], inputs["feature"]
    B = x.shape[0]
    shared = _prep_shared(inputs)

    bands = []
    for p in range(2):
        jj = np.arange(128).reshape(128, 1)
        ii = np.arange(64).reshape(1, 64)
        bands.append(np.where(jj <= 2 * ii + p, 0.0, -10000.0).astype(np.float32))

    sel = np.zeros((NT, R, 128), np.float32)
    for mi in range(NT):
        sel[mi, 2 * mi, 0:64] = 1.0
        sel[mi, 2 * mi + 1, 64:128] = 1.0
    shared["sel"] = sel

    in_maps = []
    xTs = [np.ascontiguousarray(np.asarray(x[b]).T, dtype=np.float32) for b in range(B)]
    fTs = [np.ascontiguousarray(np.asarray(feat[b]).T, dtype=np.float32) for b in range(B)]
    for core in range(NCORES):
        b, p = core // 2, core % 2
        m = dict(shared)
        m["xT"] = xTs[b]
        m["xqT"] = np.ascontiguousarray(xTs[b][:, p::2])
        m["fT"] = fTs[b]
        m["band"] = bands[p]
        in_maps.append(m)
    return in_maps, B


def kernel(**inputs):
    from concourse.bass_utils import run_bass_kernel_spmd

    nc = _get_program()
    in_maps, B = _make_in_maps(inputs)
    res = run_bass_kernel_spmd(nc, in_maps, core_ids=list(range(NCORES)))
    out = np.zeros((B, T, C), np.float32)
    for core in range(NCORES):
        b, p = core // 2, core % 2
        out[b, p::2, :] = np.asarray(res.results[core]["outT"], dtype=np.float32).T
    return out



# revision 20
# speedup vs baseline: 1.4047x; 1.4047x over previous
"""Trainium2 Bass kernel for nn_Block_with_lora (dense transformer block).

Sharding: 8 cores = 4 batches x 2 token-parity shards (stride-2 over T).
Each core computes its 512 query tokens end-to-end (no collectives);
K/V projections over all 1024 tokens are computed per-core.

Host-side prep folds LoRA (W + s*B*A) and the LayerNorm affine (gamma into
weight columns, W@beta into bias) into the dense weights, so the device
runs pure GEMMs. LayerNorm itself is applied via a rank-2 correction
matmul (colsum(W) x (-mean*rstd) + bias x std) accumulated into each
projection PSUM plus a per-token rstd multiply at drain time, so GEMMs
consume raw bf16 activations and never wait on normalized tiles.

Attention: scores via row-tiled matmul pairs (two heads concurrently in
the 128x128 PE array, K=64 each), additive causal band, Exp on Scalar,
AV matmul with an extra ones-column of V accumulating the softmax
denominator; per-head-pair epilogue broadcasts the denominator row with a
K=1 ones-matmul and normalizes with a DVE divide.
"""

import os
import sys

sys.path.insert(0, "/opt/trn_rl_repo")

import numpy as np
import ml_dtypes
from contextlib import ExitStack

BF = ml_dtypes.bfloat16

C = 1024
H = 16
DH = 64
R = 16
SCALE = 1.0 / R
T = 1024
TQ = 512
NT = 8  # C / 128
EPS = 1e-5
NCORES = 8

_PROG = None


def _build_program():
    import concourse.bass as bass
    import concourse.tile as tile
    from concourse import mybir, bacc

    f32 = mybir.dt.float32
    bf16 = mybir.dt.bfloat16
    AF = mybir.ActivationFunctionType
    AL = mybir.AluOpType

    nc = bacc.Bacc("TRN2", target_bir_lowering=False, debug=False)

    def din(name, shape, dt=f32):
        return nc.dram_tensor(name, shape, dt, kind="ExternalInput").ap()

    xbT_d = din("xbT", [C, T], bf16)
    xqT_d = din("xqT", [C, TQ])
    fbT_d = din("fbT", [C, T], bf16)
    band_d = din("band", [128, 64])

    w_d = {}
    for n in ["wq", "wk", "wv", "wsp", "wcq", "wck", "wcv", "wcp"]:
        w_d[n] = din(n, [C, C], bf16)
    w_d["wfc"] = din("wfc", [C, 4 * C], bf16)
    w_d["wpr"] = din("wpr", [4 * C, C], bf16)
    aux_d = {n: din(n, [2, C], bf16) for n in ["aux_q", "aux_k", "aux_v", "aux_cq"]}
    bias_d = {n: din(n, [C], f32) for n in ["bsp", "bck", "bcp", "bpr"]}
    bias_d["bfc"] = din("bfc", [4 * C], f32)
    bcvrow_d = din("bcv_row", [1, C], bf16)

    outT_d = nc.dram_tensor("outT", [C, TQ], f32, kind="ExternalOutput").ap()
    KDBG = os.environ.get("KDBG", "") == "1"
    dbg_d = {}
    if KDBG:
        for n, shp, dt in [("d_k2T", [C, T], bf16), ("d_rows2", [2, T], bf16),
                           ("d_rbf", [128, T], f32), ("d_qT", [C, TQ], bf16),
                           ("d_kT", [C, T], bf16), ("d_v", [C, 1040], bf16),
                           ("d_oT", [C, TQ], bf16), ("d_r1", [C, TQ], f32),
                           ("d_q2T", [C, TQ], bf16), ("d_o2T", [C, TQ], bf16),
                           ("d_r2", [C, TQ], f32), ("d_ln2", [C, TQ], bf16),
                           ("d_m", [2 * C, 1024], bf16)]:
            dbg_d[n] = nc.dram_tensor(n, shp, dt, kind="ExternalOutput").ap()

    with tile.TileContext(nc) as tc, ExitStack() as ctx:

        def pool(name, bufs, space=None):
            kw = dict(name=name, bufs=bufs)
            if space:
                kw["space"] = space
            return ctx.enter_context(tc.tile_pool(**kw))

        # SBUF pools
        actbig = pool("actbig", 16)   # [128,1024] bf16: xb(8)+fb(8) -> m_sb(16)
        kpool = pool("kpool", 8)      # [128,1024] bf16: kT
        k2pool = pool("k2pool", 8)    # [128,1024] bf16: k2T
        vpool = pool("vpool", 16)     # [128,1040] bf16: vt(8)+v2t(8)
        qpool = pool("qpool", 8)      # [128,512] bf16: qT -> q2T -> ln2
        opool = pool("opool", 8)      # [128,512] bf16: oT -> o2T
        rpool = pool("rpool", 8)      # [128,512] f32: residual (persist)
        rcpool = pool("rcpool", 8)    # [128,512] bf16: resid casts x3 gens
        wpool = pool("wpool", 6)      # [128,1024] bf16 weight chunks
        epool = pool("epool", 3)      # [128,1024] bf16: squares / exp(S)
        sbigT = pool("sbigT", 1)      # [128,1024] f32: rb bcast full-T
        sbigS = pool("sbigS", 2)      # [128,512] f32: small LN bcasts
        rows = pool("rows", 3)        # [1,1024] f32 stat rows (full T)
        srows = pool("srows", 3)      # [1,512] f32 stat rows (own)
        rbfp = pool("rbfp", 2)        # [1,<=1024] bf16 std rows
        rows2p = pool("rows2p", 1)    # [2,1024] bf16 correction rows
        rows2sp = pool("rows2sp", 2)  # [2,512] bf16 correction rows (own)
        rrp = pool("rrp", 2)          # [1,512] bf16 softmax denom rows
        rcolp = pool("rcolp", 1)      # [128,8] f32 rstd col-packed
        auxp = pool("auxp", 1)        # [2,1024] bf16 aux tensors
        smalls = pool("smalls", 1)    # [128,<=32] bias columns (per tag)
        onesp = pool("onesp", 1)
        bandp = pool("bandp", 1)
        bvp = pool("bvp", 1)

        # PSUM pools: 4 + 2 + 2 = 8 banks
        ps = pool("ps", 2, space="PSUM")   # [128,1024] f32: scores / stats / pr
        po = pool("po", 2, space="PSUM")   # [<=128,512] f32: attn out / proj
        pp = pool("pp", 2, space="PSUM")   # [128,512] f32: proj / denb

        # ---- constants ----
        ones_c16 = onesp.tile([128, 1], bf16, tag="oc16")
        nc.gpsimd.memset(ones_c16[:], 1.0)
        ones_r16 = onesp.tile([1, 128], bf16, tag="or16")
        nc.gpsimd.memset(ones_r16[:], 1.0)
        ones_r32 = onesp.tile([1, 128], f32, tag="or32")
        nc.gpsimd.memset(ones_r32[:], 1.0)
        eps_t = onesp.tile([1, 1], f32, tag="eps")
        nc.gpsimd.memset(eps_t[:], EPS)
        one_1x1 = onesp.tile([1, 1], f32, tag="one11")
        nc.gpsimd.memset(one_1x1[:], 1.0)
        dum = onesp.tile([1, 8], f32, tag="dum")
        nc.gpsimd.memset(dum[:], 1.0)
        # prime the ln+exp activation table before anything depends on it
        nc.scalar.activation(dum[:], dum[:], AF.Ln, bias=eps_t[:])
        nc.scalar.activation(dum[:], dum[:], AF.Exp)

        band_t = bandp.tile([128, 64], f32, tag="band")
        nc.scalar.dma_start(band_t[:], band_d[:, :])

        def load_percol(name, n=NT):
            t = smalls.tile([128, n], f32, tag=name)
            nc.scalar.dma_start(t[:], bias_d[name].rearrange("(m p) -> p m", p=128))
            return t

        bias_t = {n: load_percol(n) for n in ["bsp", "bck", "bcp", "bpr"]}
        bias_t["bfc"] = load_percol("bfc", 32)
        bcv_t = bvp.tile([1, C], bf16, tag="bcv")
        nc.scalar.dma_start(bcv_t[:], bcvrow_d[:, :])
        aux_t = {}
        for n in ["aux_q", "aux_k", "aux_v", "aux_cq"]:
            a = auxp.tile([2, C], bf16, tag=n)
            nc.scalar.dma_start(a[:], aux_d[n][:, :])
            aux_t[n] = a

        # ---- activation loads (gpsimd queue; weights go on sync) ----
        fb = [actbig.tile([128, T], bf16, tag="actbig", name=f"fb{i}") for i in range(NT)]
        for k in range(NT):
            nc.gpsimd.dma_start(fb[k][:], fbT_d[k * 128:(k + 1) * 128, :])
        xb = [actbig.tile([128, T], bf16, tag="actbig", name=f"xb{i}") for i in range(NT)]
        for k in range(NT):
            nc.gpsimd.dma_start(xb[k][:], xbT_d[k * 128:(k + 1) * 128, :])
        resid = []
        for k in range(NT):
            rt = rpool.tile([128, TQ], f32, tag="rpool")
            nc.gpsimd.dma_start(rt[:], xqT_d[k * 128:(k + 1) * 128, :])
            resid.append(rt)

        # =============== helpers ===============
        def wload(wname, kk, mh, colbase=0):
            """[128,1024] tile holding k-blocks (2kk,2kk+1) of a 512-col half."""
            wt = wpool.tile([128, 1024], bf16, tag="wpool")
            src = w_d[wname][2 * kk * 128:(2 * kk + 2) * 128,
                             colbase + mh * 512:colbase + (mh + 1) * 512]
            nc.sync.dma_start(
                wt[:].rearrange("p (j f) -> p j f", f=512),
                src.rearrange("(j p) f -> p j f", p=128))
            return wt

        def dense_proj(wname, rhs_tiles, Tn, drain, corr=None, pools=None):
            """out^T[mi] tiles via PE; optional K=2 LN-correction matmul.

            corr = (aux_tile, rows2_tile) accumulated as aux[:,mi]^T @ rows2.
            drain(mi, h, pt) consumes each [128,512] psum.
            """
            if pools is None:
                pools = ((pp, "pp"), (po, "po"))
            pcnt = 0
            for mh in range(2):
                wts = [wload(wname, kk, mh) for kk in range(4)]
                for ml in range(4):
                    mi = mh * 4 + ml
                    for h in range(Tn // 512):
                        sl = slice(h * 512, (h + 1) * 512)
                        pl, ptag = pools[pcnt % len(pools)]
                        pcnt += 1
                        pt = pl.tile([128, 512], f32, tag=ptag)
                        for k in range(NT):
                            kk, j = k // 2, k % 2
                            nc.tensor.matmul(
                                pt[:], wts[kk][:, j * 512 + ml * 128:j * 512 + (ml + 1) * 128],
                                rhs_tiles[k][:, sl], start=(k == 0),
                                stop=(k == NT - 1 and corr is None))
                        if corr is not None:
                            a_t, r2 = corr
                            nc.tensor.matmul(pt[:], a_t[:, mi * 128:(mi + 1) * 128],
                                             r2[:, sl], start=False, stop=True)
                        drain(mi, h, pt)

        def dense_projV(wname, lhs_tiles, v_tiles, corr=None, bias_row=None,
                        rstd_col=None, pools=None):
            """V natural [tok, dim]: activations stationary, weights moving."""
            pcnt = 0
            if pools is None:
                pools = ((pp, "pp"), (po, "po"))
            for dh in range(2):
                sl = slice(dh * 512, (dh + 1) * 512)
                wts = [wload(wname, kk, dh) for kk in range(4)]
                for tt in range(NT):
                    pl, ptag = pools[pcnt % len(pools)]
                    pcnt += 1
                    pt = pl.tile([128, 512], f32, tag=ptag)
                    for k in range(NT):
                        kk, j = k // 2, k % 2
                        nc.tensor.matmul(
                            pt[:], lhs_tiles[k][:, tt * 128:(tt + 1) * 128],
                            wts[kk][:, j * 512:(j + 1) * 512], start=(k == 0), stop=False)
                    if corr is not None:
                        a_t, r2 = corr
                        nc.tensor.matmul(pt[:], r2[:, tt * 128:(tt + 1) * 128],
                                         a_t[:, sl], start=False, stop=True)
                    else:
                        nc.tensor.matmul(pt[:], ones_r16[:], bias_row[:, sl],
                                         start=False, stop=True)
                    dest = v_tiles[tt][:, dh * 520:(dh + 1) * 520]
                    dest = dest.rearrange("p (h d) -> p h d", d=65)[:, :, 0:64]
                    if rstd_col is not None:
                        nc.vector.tensor_scalar_mul(dest, pt[:], rstd_col[:, tt:tt + 1])
                    else:
                        nc.vector.tensor_copy(dest, pt[:])

        def ln_rows(xb_tiles, sq_tiles, Tn, rows_pool, rows2_tile, rb_tile,
                    rstd_col=None, negmr_out=None):
            """Stats over channel dim -> rows2 [2,Tn] (-m*r, std), rb bcast."""
            nh = Tn // 512
            mean_ps = ps.tile([1, Tn], f32, tag="ps")
            sq_ps = ps.tile([1, Tn], f32, tag="ps")
            for k in range(NT):
                for hh in range(nh):
                    sl = slice(hh * 512, (hh + 1) * 512)
                    nc.tensor.matmul(mean_ps[0:1, sl], ones_c16[:], xb_tiles[k][:, sl],
                                     start=(k == 0), stop=(k == NT - 1))
                    nc.tensor.matmul(sq_ps[0:1, sl], ones_c16[:], sq_tiles[k][:, sl],
                                     start=(k == 0), stop=(k == NT - 1))
            mean_row = rows_pool.tile([1, Tn], f32, tag="r")
            var_row = rows_pool.tile([1, Tn], f32, tag="r")
            rstd_row = rows_pool.tile([1, Tn], f32, tag="r")
            nc.vector.tensor_scalar_mul(mean_row[:], mean_ps[:], 1.0 / C)
            nc.vector.tensor_mul(var_row[:], mean_row[:], mean_row[:])
            nc.vector.scalar_tensor_tensor(var_row[:], sq_ps[:], 1.0 / C, var_row[:],
                                           op0=AL.mult, op1=AL.subtract)
            # rstd = exp(-0.5*ln(var+eps)); std = exp(+0.5*ln(var+eps))
            nc.scalar.activation(var_row[:], var_row[:], AF.Ln, bias=eps_t[:])
            nc.scalar.activation(rstd_row[:], var_row[:], AF.Exp, scale=-0.5)
            if rows2_tile is not None:
                # rows2: row0 = -mean*rstd (bf16), row1 = std (bf16 via DMA)
                std_bf = rbfp.tile([1, Tn], bf16, tag="rbf")
                nc.scalar.activation(std_bf[:], var_row[:], AF.Exp, scale=0.5)
                nc.vector.tensor_scalar_mul(rows2_tile[0:1, :], mean_row[:], -1.0)
                nc.scalar.dma_start(rows2_tile[1:2, :], std_bf[:])
            # rstd broadcast [128,Tn] f32 via K=1 matmul
            for hh in range(nh):
                sl = slice(hh * 512, (hh + 1) * 512)
                bp = pp.tile([128, 512], f32, tag="pp")
                nc.tensor.matmul(bp[:], ones_r32[:], rstd_row[0:1, sl],
                                 start=True, stop=True)
                nc.vector.tensor_copy(rb_tile[:, sl], bp[:])
            if rstd_col is not None:
                # transpose rstd row into per-token-block columns via K=1 MMs
                rcps = pp.tile([128, NT], f32, tag="pp")
                for tt in range(NT):
                    nc.tensor.matmul(rcps[:, tt:tt + 1],
                                     rstd_row[0:1, tt * 128:(tt + 1) * 128],
                                     one_1x1[:], start=True, stop=True)
                nc.vector.tensor_copy(rstd_col[:], rcps[:])
            if negmr_out is not None:
                # broadcast of -mean*rstd for explicit normalize
                nc.vector.scalar_tensor_tensor(var_row[:], mean_row[:], -1.0,
                                               rstd_row[:], op0=AL.mult, op1=AL.mult)
                bp = pp.tile([128, 512], f32, tag="pp")
                nc.tensor.matmul(bp[:], ones_r32[:], var_row[0:1, :],
                                 start=True, stop=True)
                nc.vector.tensor_copy(negmr_out[:], bp[:])

        def attention(q_tiles, k_tiles, v_tiles, o_tiles):
            for mi in range(NT):
                opA = po.tile([65, 512], f32, tag="po", name=f"opA{mi}")
                opB = po.tile([65, 512], f32, tag="po", name=f"opB{mi}")
                hA, hB = 2 * mi, 2 * mi + 1
                for jp in range(4):
                    kj0, kj1 = 2 * jp, 2 * jp + 1
                    q0, q1 = 64 * kj0, 64 * kj1
                    e1 = 512 + (512 - q1)
                    stA = ps.tile([128, 1024], f32, tag="ps")
                    stB = ps.tile([128, 1024], f32, tag="ps")
                    for st, off in ((stA, 0), (stB, 64)):
                        nc.tensor.matmul(
                            st[:, q0:512],
                            k_tiles[mi][off:off + 64, kj0 * 128:(kj0 + 1) * 128],
                            q_tiles[mi][off:off + 64, q0:512], start=True, stop=True)
                        nc.tensor.matmul(
                            st[:, 512:e1],
                            k_tiles[mi][off:off + 64, kj1 * 128:(kj1 + 1) * 128],
                            q_tiles[mi][off:off + 64, q1:512], start=True, stop=True)
                    for st in (stA, stB):
                        nc.vector.tensor_add(st[:, q0:q0 + 64], st[:, q0:q0 + 64], band_t[:])
                        nc.vector.tensor_add(st[:, 512:576], st[:, 512:576], band_t[:])
                    etA = epool.tile([128, 1024], bf16, tag="e")
                    etB = epool.tile([128, 1024], bf16, tag="e")
                    nc.scalar.activation(etA[:, q0:e1], stA[:, q0:e1], AF.Exp)
                    nc.scalar.activation(etB[:, q0:e1], stB[:, q0:e1], AF.Exp)
                    for op_, et, hh in ((opA, etA, hA), (opB, etB, hB)):
                        nc.tensor.matmul(
                            op_[:] if kj0 == 0 else op_[:, q0:512],
                            v_tiles[kj0][:, 65 * hh:65 * hh + 65],
                            et[:, q0:512], start=(kj0 == 0), stop=False)
                        nc.tensor.matmul(
                            op_[:, q1:512],
                            v_tiles[kj1][:, 65 * hh:65 * hh + 65],
                            et[:, 512:e1], start=False, stop=(kj1 == 7))
                # epilogue: reciprocal denom rows, f32r broadcast, multiply
                rrA = rrp.tile([1, 512], f32, tag="rr")
                rrB = rrp.tile([1, 512], f32, tag="rr")
                nc.vector.tensor_copy(rrA[:], opA[64:65, :])
                nc.vector.tensor_copy(rrB[:], opB[64:65, :])
                nc.vector.reciprocal_approx_fast(rrA[:], rrA[:])
                nc.vector.reciprocal_approx_fast(rrB[:], rrB[:])
                rrAb = rrp.tile([1, 512], bf16, tag="rrb")
                rrBb = rrp.tile([1, 512], bf16, tag="rrb")
                nc.vector.tensor_copy(rrAb[:], rrA[:])
                nc.vector.tensor_copy(rrBb[:], rrB[:])
                denb = pp.tile([128, 512], f32, tag="pp")
                nc.tensor.matmul(denb[0:64, :], ones_r16[0:1, 0:64], rrAb[:],
                                 start=True, stop=True)
                nc.tensor.matmul(denb[64:128, :], ones_r16[0:1, 0:64], rrBb[:],
                                 start=True, stop=True)
                den_sb = sbigS.tile([128, 512], f32, tag="sbS")
                nc.vector.tensor_copy(den_sb[:], denb[:])
                nc.vector.tensor_mul(o_tiles[mi][0:64, :], opA[0:64, :],
                                     den_sb[0:64, :])
                nc.vector.tensor_mul(o_tiles[mi][64:128, :], opB[0:64, :],
                                     den_sb[64:128, :])

        def dump(name, tiles, rows=128):
            if not KDBG:
                return
            for i, t in enumerate(tiles):
                nc.gpsimd.dma_start(dbg_d[name][i * rows:(i + 1) * rows, :], t[:])

        # =============== phase 1: cross-K GEMM (raw feature) ===============
        k2T = [k2pool.tile([128, T], bf16, tag="k2", name=f"k2T{i}") for i in range(NT)]

        def ck_drain(mi, h, pt):
            nc.vector.tensor_scalar_add(k2T[mi][:, h * 512:(h + 1) * 512], pt[:],
                                        bias_t["bck"][:, mi:mi + 1])

        dense_proj("wck", fb, T, ck_drain)
        dump("d_k2T", k2T)

        # =============== phase 2: LN1 stats over full x ===============
        sqx = []
        for k in range(NT):
            sq = epool.tile([128, T], bf16, tag="e")
            nc.vector.tensor_mul(sq[:], xb[k][:], xb[k][:])
            sqx.append(sq)
        rows2 = rows2p.tile([2, T], bf16, tag="r2")
        rb_f = sbigT.tile([128, T], f32, tag="sbT")
        rcol = rcolp.tile([128, NT], f32, tag="rcol")
        ln_rows(xb, sqx, T, rows, rows2, rb_f, rstd_col=rcol)
        dump("d_rows2", [rows2], rows=2)
        dump("d_rbf", [rb_f])

        # own-token stats from resid
        rc1 = [rcpool.tile([128, TQ], bf16, tag="rc", name=f"rc1_{i}") for i in range(NT)]
        sq1 = []
        for k in range(NT):
            nc.vector.tensor_copy(rc1[k][:], resid[k][:])
            sq = epool.tile([128, TQ], bf16, tag="e")
            nc.vector.tensor_mul(sq[:], rc1[k][:], rc1[k][:])
            sq1.append(sq)
        rows2o = rows2sp.tile([2, TQ], bf16, tag="r2s")
        rb_o = sbigS.tile([128, TQ], f32, tag="sbS")
        ln_rows(rc1, sq1, TQ, srows, rows2o, rb_o)

        # =============== phase 3: self q/k/v GEMMs ===============
        qT = [qpool.tile([128, TQ], bf16, tag="q", name=f"qT{i}") for i in range(NT)]

        def q_drain(mi, h, pt):
            nc.vector.tensor_mul(qT[mi][:], pt[:], rb_o[:])

        dense_proj("wq", rc1, TQ, q_drain, corr=(aux_t["aux_q"], rows2o))

        kT = [kpool.tile([128, T], bf16, tag="k", name=f"kT{i}") for i in range(NT)]

        def k_drain(mi, h, pt):
            sl = slice(h * 512, (h + 1) * 512)
            nc.vector.tensor_mul(kT[mi][:, sl], pt[:], rb_f[:, sl])

        dense_proj("wk", xb, T, k_drain, corr=(aux_t["aux_k"], rows2))

        vt = [vpool.tile([128, 1040], bf16, tag="v", name=f"vt{i}") for i in range(NT)]
        for tt in range(NT):
            nc.gpsimd.memset(vt[tt][:, 64:1040:65], 1.0)
        dense_projV("wv", xb, vt, corr=(aux_t["aux_v"], rows2), rstd_col=rcol)
        dump("d_qT", qT)
        dump("d_kT", kT)
        dump("d_v", vt)

        # =============== phase 4: self attention ===============
        oT = [opool.tile([128, TQ], bf16, tag="o", name=f"oT{i}") for i in range(NT)]
        attention(qT, kT, vt, oT)
        dump("d_oT", oT)

        # =============== phase 5: cross-V GEMM (scheduler fills attn gaps) ===
        v2t = [vpool.tile([128, 1040], bf16, tag="v", name=f"v2t{i}") for i in range(NT)]
        for tt in range(NT):
            nc.gpsimd.memset(v2t[tt][:, 64:1040:65], 1.0)
        dense_projV("wcv", fb, v2t, bias_row=bcv_t, pools=((pp, "pp"),))

        # =============== phase 6: self proj + residual ===============
        def sp_drain(mi, h, pt):
            nc.vector.scalar_tensor_tensor(resid[mi][:], pt[:], bias_t["bsp"][:, mi:mi + 1],
                                           resid[mi][:], op0=AL.add, op1=AL.add)

        dense_proj("wsp", oT, TQ, sp_drain)
        dump("d_r1", resid)

        # =============== phase 7: LN1 on updated own tokens ===============
        rc2 = [rcpool.tile([128, TQ], bf16, tag="rc", name=f"rc2_{i}") for i in range(NT)]
        sq2 = []
        for k in range(NT):
            nc.vector.tensor_copy(rc2[k][:], resid[k][:])
            sq = epool.tile([128, TQ], bf16, tag="e")
            nc.vector.tensor_mul(sq[:], rc2[k][:], rc2[k][:])
            sq2.append(sq)
        rows2o2 = rows2sp.tile([2, TQ], bf16, tag="r2s")
        rb_o2 = sbigS.tile([128, TQ], f32, tag="sbS")
        ln_rows(rc2, sq2, TQ, srows, rows2o2, rb_o2)

        # =============== phase 8: cross q ===============
        q2T = [qpool.tile([128, TQ], bf16, tag="q", name=f"q2T{i}") for i in range(NT)]

        def q2_drain(mi, h, pt):
            nc.vector.tensor_mul(q2T[mi][:], pt[:], rb_o2[:])

        dense_proj("wcq", rc2, TQ, q2_drain, corr=(aux_t["aux_cq"], rows2o2))
        dump("d_q2T", q2T)

        # =============== phase 9: cross attention ===============
        o2T = [opool.tile([128, TQ], bf16, tag="o", name=f"o2T{i}") for i in range(NT)]
        attention(q2T, k2T, v2t, o2T)
        dump("d_o2T", o2T)
        # prime the gelu table while cp runs
        nc.scalar.activation(dum[:], dum[:], AF.Gelu_apprx_tanh)

        # =============== phase 10: cross proj + residual ===============
        def cp_drain(mi, h, pt):
            nc.vector.scalar_tensor_tensor(resid[mi][:], pt[:], bias_t["bcp"][:, mi:mi + 1],
                                           resid[mi][:], op0=AL.add, op1=AL.add)

        dense_proj("wcp", o2T, TQ, cp_drain)
        dump("d_r2", resid)

        # =============== phase 11: LN2 (explicit normalize) ===============
        rc3 = [rcpool.tile([128, TQ], bf16, tag="rc", name=f"rc3_{i}") for i in range(NT)]
        sq3 = []
        for k in range(NT):
            nc.vector.tensor_copy(rc3[k][:], resid[k][:])
            sq = epool.tile([128, TQ], bf16, tag="e")
            nc.vector.tensor_mul(sq[:], rc3[k][:], rc3[k][:])
            sq3.append(sq)
        rb_o3 = sbigS.tile([128, TQ], f32, tag="sbS")
        nmr_o3 = sbigS.tile([128, TQ], f32, tag="sbS")
        ln_rows(rc3, sq3, TQ, srows, None, rb_o3, negmr_out=nmr_o3)
        ln2 = [qpool.tile([128, TQ], bf16, tag="q", name=f"ln2_{i}") for i in range(NT)]
        for k in range(NT):
            nc.vector.tensor_mul(ln2[k][:], resid[k][:], rb_o3[:])
            nc.vector.tensor_add(ln2[k][:], ln2[k][:], nmr_o3[:])

        dump("d_ln2", ln2)

        # =============== phase 12: MLP fc ===============
        m_sb = [actbig.tile([128, 1024], bf16, tag="actbig", name=f"m{i}")
                for i in range(16)]
        pcnt = 0
        fpools = ((pp, "pp"), (po, "po"))
        for g in range(8):
            wts = [wload("wfc", kk, 0, colbase=g * 512) for kk in range(4)]
            for ml in range(4):
                mi = g * 4 + ml
                pl, ptag = fpools[pcnt % 2]
                pcnt += 1
                pt = pl.tile([128, 512], f32, tag=ptag)
                for k in range(NT):
                    kk, j = k // 2, k % 2
                    nc.tensor.matmul(pt[:], wts[kk][:, j * 512 + ml * 128:j * 512 + (ml + 1) * 128],
                                     ln2[k][:], start=(k == 0), stop=(k == NT - 1))
                nc.scalar.activation(m_sb[mi // 2][:, (mi % 2) * 512:(mi % 2 + 1) * 512],
                                     pt[:], AF.Gelu_apprx_tanh,
                                     bias=bias_t["bfc"][:, mi:mi + 1])

        dump("d_m", m_sb)

        # =============== phase 13: MLP proj, k-outer over all 8 psum banks ===
        psA = ps.tile([128, 1024], f32, tag="ps")
        psB = ps.tile([128, 1024], f32, tag="ps")
        poA = po.tile([128, 512], f32, tag="po")
        poB = po.tile([128, 512], f32, tag="po")
        ppA = pp.tile([128, 512], f32, tag="pp")
        ppB = pp.tile([128, 512], f32, tag="pp")
        prq = [psA[:, 0:512], psA[:, 512:1024], psB[:, 0:512], psB[:, 512:1024],
               poA[:], poB[:], ppA[:], ppB[:]]
        for k in range(32):
            wt = wpool.tile([128, 1024], bf16, tag="wpool")
            nc.sync.dma_start(wt[:], w_d["wpr"][k * 128:(k + 1) * 128, :])
            ms = m_sb[k // 2][:, (k % 2) * 512:(k % 2 + 1) * 512]
            for j in range(8):
                nc.tensor.matmul(prq[j], wt[:, j * 128:(j + 1) * 128], ms,
                                 start=(k == 0), stop=(k == 31))
        for j in range(8):
            nc.vector.scalar_tensor_tensor(resid[j][:], prq[j], bias_t["bpr"][:, j:j + 1],
                                           resid[j][:], op0=AL.add, op1=AL.add)
            nc.gpsimd.dma_start(outT_d[j * 128:(j + 1) * 128, :], resid[j][:])

    nc.compile()
    return nc


def _get_program():
    global _PROG
    if _PROG is None:
        _PROG = _build_program()
    return _PROG


def _prep_shared(inputs):
    g = {}

    def bf(a):
        return np.ascontiguousarray(np.asarray(a, dtype=np.float32)).astype(BF)

    def f(a):
        return np.ascontiguousarray(np.asarray(a, dtype=np.float32))

    inv = 1.0 / np.sqrt(DH)
    g1, b1 = np.asarray(inputs["ln1_g"]), np.asarray(inputs["ln1_b"])
    g2, b2 = np.asarray(inputs["ln2_g"]), np.asarray(inputs["ln2_b"])

    def fold(w, b, a, lb, ln=None):
        W = np.asarray(w, np.float32) + SCALE * (np.asarray(lb, np.float32)
                                                 @ np.asarray(a, np.float32))
        beff = np.asarray(b, np.float32).copy()
        if ln is not None:
            gg, bb = ln
            beff = beff + W @ bb
            W = W * gg[None, :]
        return W, beff

    def aux_of(WT, beff):
        # rows: colsum (for -m*r term), bias (times std term)
        return bf(np.stack([WT.sum(axis=0), beff], axis=0))

    Wqkv, bqkv = fold(inputs["sa_qkv_w"], inputs["sa_qkv_b"],
                      inputs["sa_qkv_a"], inputs["sa_qkv_lb"], ln=(g1, b1))
    qw, kw, vw = (Wqkv[i * C:(i + 1) * C] for i in range(3))
    qb, kb, vb = (bqkv[i * C:(i + 1) * C] for i in range(3))
    g["wq"] = bf(qw.T * inv)
    g["wk"] = bf(kw.T)
    g["wv"] = bf(vw.T)
    g["aux_q"] = aux_of(qw.T * inv, qb * inv)
    g["aux_k"] = aux_of(kw.T, kb)
    g["aux_v"] = aux_of(vw.T, vb)

    Wsp, bsp = fold(inputs["sa_proj_w"], inputs["sa_proj_b"],
                    inputs["sa_proj_a"], inputs["sa_proj_lb"])
    g["wsp"] = bf(Wsp.T)
    g["bsp"] = f(bsp)

    Wcq, bcq = fold(inputs["ca_q_w"], inputs["ca_q_b"],
                    inputs["ca_q_a"], inputs["ca_q_lb"], ln=(g1, b1))
    g["wcq"] = bf(Wcq.T * inv)
    g["aux_cq"] = aux_of(Wcq.T * inv, bcq * inv)

    Wckv, bckv = fold(inputs["ca_kv_w"], inputs["ca_kv_b"],
                      inputs["ca_kv_a"], inputs["ca_kv_lb"])
    g["wck"] = bf(Wckv[0:C].T)
    g["wcv"] = bf(Wckv[C:2 * C].T)
    g["bck"] = f(bckv[0:C])
    g["bcv_row"] = bf(bckv[C:2 * C].reshape(1, C))

    Wcp, bcp = fold(inputs["ca_proj_w"], inputs["ca_proj_b"],
                    inputs["ca_proj_a"], inputs["ca_proj_lb"])
    g["wcp"] = bf(Wcp.T)
    g["bcp"] = f(bcp)

    Wfc = np.asarray(inputs["fc_w"], np.float32) * g2[None, :]
    bfc = np.asarray(inputs["fc_b"], np.float32) + np.asarray(inputs["fc_w"], np.float32) @ b2
    g["wfc"] = bf(Wfc.T)
    g["bfc"] = f(bfc)
    g["wpr"] = bf(np.asarray(inputs["pr_w"]).T)
    g["bpr"] = f(inputs["pr_b"])
    return g


def _make_in_maps(inputs):
    inputs = {k: np.asarray(v) for k, v in inputs.items()}
    x, feat = inputs["x"], inputs["feature"]
    B = x.shape[0]
    shared = _prep_shared(inputs)

    bands = []
    for p in range(2):
        jj = np.arange(128).reshape(128, 1)
        ii = np.arange(64).reshape(1, 64)
        bands.append(np.where(jj <= 2 * ii + p, 0.0, -10000.0).astype(np.float32))

    in_maps = []
    xTs = [np.ascontiguousarray(np.asarray(x[b]).T, dtype=np.float32) for b in range(B)]
    fTs = [np.ascontiguousarray(np.asarray(feat[b]).T, dtype=np.float32) for b in range(B)]
    for core in range(NCORES):
        b, p = core // 2, core % 2
        m = dict(shared)
        m["xbT"] = xTs[b].astype(BF)
        m["xqT"] = np.ascontiguousarray(xTs[b][:, p::2])
        m["fbT"] = fTs[b].astype(BF)
        m["band"] = bands[p]
        in_maps.append(m)
    return in_maps, B


def kernel(**inputs):
    from concourse.bass_utils import run_bass_kernel_spmd

    nc = _get_program()
    in_maps, B = _make_in_maps(inputs)
    res = run_bass_kernel_spmd(nc, in_maps, core_ids=list(range(NCORES)))
    out = np.zeros((B, T, C), np.float32)
    for core in range(NCORES):
        b, p = core // 2, core % 2
        out[b, p::2, :] = np.asarray(res.results[core]["outT"], dtype=np.float32).T
    return out
